# revision 3
# baseline (speedup 1.0000x reference)
"""Trainium2 Bass kernel for the Human3.6M pose postprocess
(spherical->xyz conversion + kinematic-tree accumulation).

Self-contained: hardcodes the problem shapes
  observed_pose (4096, 16, 96) f32, pred_pose (4096, 64, 66) f32
and shards the batch dim across 8 NeuronCores (pure data parallel).

Per-core design (bt-major layout, no transposes, no TensorE):
  - partition p <-> one batch example; free dim = (64 time steps) x channels
  - range reduction to [-pi, pi] without a mod op:
      n  = round(x / 2pi)   via two ACT affine passes (big-constant
                            round-to-nearest trick: +C then -C, C = 1.5*2^23)
      w  = x - 2pi*n        one DVE scalar_tensor_tensor
    theta and phi are processed interleaved (one strided op covers both).
  - sin = Sin(w); cos = Sin(pi/2 - |w|)  (both args within the +-4 LUT range)
  - DVE: muls for spherical->xyz, then ONE gated tensor_tensor_scan per xyz
    component computes the entire 22-edge kinematic tree walk in natural
    output-channel order:  state = gate*state + w;  gate=0 at root channels
    {0,1,6,11} reseeds state from the last observed frame, zero-offset slots
    reproduce the IGNORE copies, and two correction slots (ch16, ch24)
    rewind the state to x[13] across tree branches.
Output leaves the device component-major [nb, 3, 64, 32]; the host
transposes back to (B, T, 96).
"""

import math
import sys

for _p in ("/opt/trn_rl_repo", "/root/.axon_site/_ro/trn_rl_repo"):
    if _p not in sys.path:
        sys.path.insert(0, _p)

import numpy as np

PI = math.pi
BIGC = 1.5 * 2**23  # fp32 round-to-nearest-integer constant
T = 64   # time steps = reps per partition
P = 128  # partitions per tile

N_CORES = 8
B = 4096
NB = B // N_CORES  # batches per core

# child-joint order of CONNECT (k index) -> contiguous runs in output-channel
# space: (k_start, ch_start, length)
ASSEM_RUNS = [
    (0, 12, 4),   # k0..3   -> ch12..15  (spine 12,13,14,15)
    (4, 25, 3),   # k4..6   -> ch25..27  (arm 25,26,27)
    (7, 29, 2),   # k7..8   -> ch29..30  (arm 29,30)
    (9, 17, 3),   # k9..11  -> ch17..19  (arm 17,18,19)
    (12, 21, 2),  # k12..13 -> ch21..22  (arm 21,22)
    (14, 2, 4),   # k14..17 -> ch2..5    (leg 2,3,4,5)
    (18, 7, 4),   # k18..21 -> ch7..10   (leg 7,8,9,10)
]


def build_kernel(nc, n_b: int):
    """Build the postprocess kernel for n_b batch examples on one core."""
    import concourse.tile as tile
    from concourse import mybir

    f32 = mybir.dt.float32
    pred = nc.dram_tensor("pred", [n_b * T, 66], f32, kind="ExternalInput")
    obs = nc.dram_tensor("obs", [n_b, 96], f32, kind="ExternalInput")
    out = nc.dram_tensor("out", [n_b, 3 * T * 32], f32, kind="ExternalOutput")

    with tile.TileContext(nc) as tc:
        build_tile_kernel(tc, pred, obs, out, n_b)
    return nc


def build_tile_kernel(tc, pred, obs, out, n_b: int):
    import concourse.bass as bass
    from concourse import mybir

    f32 = mybir.dt.float32
    ALU = mybir.AluOpType
    ACTF = mybir.ActivationFunctionType
    nc = tc.nc
    nt = (n_b + P - 1) // P
    pp = min(P, n_b)

    # pred rows grouped per tile: partition p holds T consecutive rows (one b)
    pred_t = pred.ap().rearrange("(n p r) c -> n p (r c)", p=pp, r=T)

    with (
        tc.tile_pool(name="io", bufs=2) as io_pool,
        tc.tile_pool(name="mid", bufs=2) as mid_pool,
        tc.tile_pool(name="mid1", bufs=1) as mid1_pool,
        tc.tile_pool(name="const", bufs=1) as const_pool,
    ):
        # static gate tile [pp, T*32]: 1.0 everywhere, 0.0 at root channels
        gate = const_pool.tile([pp, T * 32], f32)
        g4 = gate.rearrange("p (r j) -> p r j", j=32)
        nc.vector.memset(gate, 1.0)
        nc.vector.memset(g4[:, :, 0:2], 0.0)       # ch 0, 1
        nc.vector.memset(g4[:, :, 6:12:5], 0.0)    # ch 6, 11

        # per-partition constant for Sin biases
        halfpi = const_pool.tile([pp, 1], f32)
        nc.vector.memset(halfpi, PI / 2)

        for i in range(nt):
            b0 = i * pp

            raw = io_pool.tile([pp, T * 66], f32)
            nc.sync.dma_start(out=raw, in_=pred_t[i])
            r4 = raw.rearrange("p (r c) -> p r c", c=66)
            # interleaved (theta, phi) strided view: offsets 1,2 of each joint
            th_ph = bass.AP(
                tensor=raw.tensor,
                offset=raw.offset + 1,
                ap=[raw.ap[0], [66, T], [3, 22], [1, 2]],
            )

            # ---- range reduction: n = round(x/2pi), w = x - 2pi*n ----
            # (in-place chain: ang ends up holding sin, absw holds cos)
            ang = mid_pool.tile([pp, T, 22, 2], f32, tag="ang")
            angf = ang.rearrange("p r k t -> p (r k t)")
            nc.scalar.activation(out=angf, in_=th_ph, func=ACTF.Copy,
                                 bias=BIGC, scale=1.0 / (2 * PI))
            nc.scalar.activation(out=angf, in_=angf, func=ACTF.Copy,
                                 bias=-BIGC, scale=1.0)
            nc.vector.scalar_tensor_tensor(
                out=angf, in0=angf, scalar=-2 * PI, in1=th_ph,
                op0=ALU.mult, op1=ALU.add,
            )
            # ---- trig: sin = Sin(w); cos = Sin(pi/2 - |w|) ----
            absw = mid_pool.tile([pp, T, 22, 2], f32, tag="absw")
            abwf = absw.rearrange("p r k t -> p (r k t)")
            nc.scalar.activation(out=abwf, in_=angf, func=ACTF.Abs)
            nc.scalar.activation(out=angf, in_=angf,
                                 func=ACTF.Sin, bias=0.0, scale=1.0)
            nc.scalar.activation(out=abwf, in_=abwf,
                                 func=ACTF.Sin, bias=halfpi[:, 0:1], scale=-1.0)
            sin_tp = ang   # [..., 0]=theta, [..., 1]=phi
            cos_tp = absw

            # ---- spherical -> xyz ----
            rsin = mid1_pool.tile([pp, T * 22], f32)
            nc.vector.tensor_mul(rsin, r4[:, :, 0:66:3],
                                 sin_tp[:, :, :, 1].rearrange("p r k -> p (r k)"))
            xyz = mid1_pool.tile([pp, 3, T, 22], f32)  # (comp, rep, k)
            x4 = xyz.rearrange("p c r k -> p c (r k)")
            # x0 = r sin(phi) cos(theta)   (DVE)
            nc.vector.tensor_mul(
                x4[:, 0], rsin,
                cos_tp[:, :, :, 0].rearrange("p r k -> p (r k)"))
            # x1 = r cos(phi)              (GPSIMD)
            nc.gpsimd.tensor_tensor(
                out=x4[:, 1], in0=r4[:, :, 0:66:3],
                in1=cos_tp[:, :, :, 1].rearrange("p r k -> p (r k)"),
                op=ALU.mult)
            # x2 = r sin(phi) sin(theta)   (GPSIMD)
            nc.gpsimd.tensor_tensor(
                out=x4[:, 2], in0=rsin,
                in1=sin_tp[:, :, :, 0].rearrange("p r k -> p (r k)"),
                op=ALU.mult)

            # ---- assemble scan work buffer W [pp, 3, T, 32] ----
            w = io_pool.tile([pp, 3, T, 32], f32)
            # zero-offset slots (ch 20,28 and 23,31 = IGNORE copies)
            nc.gpsimd.memset(w[:, :, :, 20:29:8], 0.0)
            nc.gpsimd.memset(w[:, :, :, 23:32:8], 0.0)
            # root slots from obs: ch{0,1} <- cols 0..5, ch{6,11} <- 18..20/33..35
            obs_t = mid_pool.tile([pp, 96], f32)
            nc.sync.dma_start(out=obs_t, in_=obs[b0 : b0 + pp, :])
            nc.gpsimd.tensor_copy(
                out=w[:, :, :, 0:2],
                in_=bass.AP(tensor=obs_t.tensor, offset=obs_t.offset,
                            ap=[obs_t.ap[0], [1, 3], [0, T], [3, 2]]),
            )
            nc.gpsimd.tensor_copy(
                out=w[:, :, :, 6:12:5],
                in_=bass.AP(tensor=obs_t.tensor, offset=obs_t.offset + 18,
                            ap=[obs_t.ap[0], [1, 3], [0, T], [15, 2]]),
            )
            # xyz offset slots (7 contiguous runs) on GPSIMD
            for k0, ch0, ln in ASSEM_RUNS:
                nc.gpsimd.tensor_copy(
                    out=w[:, :, :, ch0 : ch0 + ln], in_=xyz[:, :, :, k0 : k0 + ln]
                )
            # correction slots: ch16 = -(k2+k3) -> x13; ch24 = -(k9..k13) -> x13
            nc.vector.tensor_reduce(
                out=w[:, :, :, 16:17], in_=xyz[:, :, :, 2:4],
                axis=mybir.AxisListType.X, op=ALU.add, negate=True)
            nc.vector.tensor_reduce(
                out=w[:, :, :, 24:25], in_=xyz[:, :, :, 9:14],
                axis=mybir.AxisListType.X, op=ALU.add, negate=True)

            # ---- gated scan per component (in place): state = gate*state + w ----
            w2 = w.rearrange("p c r j -> p c (r j)")
            for c in range(3):
                nc.vector.tensor_tensor_scan(
                    out=w2[:, c], data0=gate, data1=w2[:, c],
                    initial=0.0, op0=ALU.mult, op1=ALU.add)

            nc.sync.dma_start(
                out=out[b0 : b0 + pp, :],
                in_=w.rearrange("p c r j -> p (c r j)"),
            )


_CACHE = {}


def _get_nc():
    if "nc" not in _CACHE:
        import concourse.bacc as bacc

        nc = bacc.Bacc("TRN2", target_bir_lowering=False)
        build_kernel(nc, NB)
        nc.compile()
        _CACHE["nc"] = nc
    return _CACHE["nc"]


def _run(in_maps, **kwargs):
    from concourse.bass_utils import run_bass_kernel_spmd

    nc = _get_nc()
    return run_bass_kernel_spmd(nc, in_maps, core_ids=list(range(N_CORES)), **kwargs)


def _make_in_maps(observed_pose, pred_pose):
    obs_last = np.ascontiguousarray(observed_pose[:, -1, :], dtype=np.float32)
    pred = np.ascontiguousarray(pred_pose, dtype=np.float32)
    in_maps = []
    for c in range(N_CORES):
        in_maps.append(
            {
                "pred": np.ascontiguousarray(
                    pred[c * NB : (c + 1) * NB].reshape(NB * T, 66)
                ),
                "obs": obs_last[c * NB : (c + 1) * NB],
            }
        )
    return in_maps


def _assemble_out(results):
    outs = []
    for c in range(N_CORES):
        o = results[c]["out"].reshape(NB, 3, T, 32)
        outs.append(o.transpose(0, 2, 3, 1).reshape(NB, T, 96))
    return np.ascontiguousarray(np.concatenate(outs, axis=0), dtype=np.float32)


def kernel(observed_pose, pred_pose):
    res = _run(_make_in_maps(observed_pose, pred_pose))
    return _assemble_out(res.results)


def kernel_traced(observed_pose, pred_pose, trace_cores=None):
    """Like kernel() but returns (output, BassKernelResults) with an NTFF trace."""
    res = _run(
        _make_in_maps(observed_pose, pred_pose),
        trace=True,
        trace_cores=trace_cores or [0],
    )
    return _assemble_out(res.results), res
